# revision 45
# baseline (speedup 1.0000x reference)
"""Causal self-attention (B=2, T=4096, C=768, H=12, D=64) on 8 trn2 cores.

Sharding: core c handles batch b = c//4 and heads [3g, 3g+3), g = c%4.
Each core computes a (4096, 768) partial of y = attn_out @ w_out restricted
to its 3 heads' rows of w_out; the host sums the 4 partials per batch.

v2: all matmul operands bf16 (f32 PSUM), x pre-transposed + pre-cast on the
host (no PE transposes), V computed token-major directly (xT chunks as
weights streaming w_v), causal column-trim on QK/exp/PV for diagonal tiles
with a single [128,128] triangular mask applied to the diagonal band on DVE,
reciprocal_approx_fast for softmax denominators.

Math per head (no max-subtraction softmax; scores are O(8) so exp is safe):
  S^T[k, q] = (K Q^T)[k, q] / 8     computed k-on-partitions
  E = exp(S^T) with causal band mask
  [Y^T; l] = [V | 1]^T E            PV matmul with a ones column -> row 64 = l
  out += (Y^T / l).T @ W_o[head rows]
"""

import os
import numpy as np
import ml_dtypes
from contextlib import ExitStack

import concourse.bass as bass
import concourse.tile as tile
from concourse import bacc, mybir
from concourse.bass_utils import run_bass_kernel_spmd

F32 = mybir.dt.float32
BF16 = mybir.dt.bfloat16

B, T, C, H, D = 2, 4096, 768, 12, 64
HPC = 3            # heads per core
NS = 8             # strips
SW = 512           # strip width (q)
KT = 128           # k tile
NKT = T // KT      # 32 k tiles
KG = 8             # k tiles per PV accumulation group


def build_program():
    nc = bacc.Bacc("TRN2", target_bir_lowering=False, debug=False, num_devices=8)

    xT_d = nc.dram_tensor("xT", [C, T], BF16, kind="ExternalInput").ap()
    wqk_d = nc.dram_tensor("wqk", [C, 384], BF16, kind="ExternalInput").ap()
    wv_d = nc.dram_tensor("wv", [C, 192], BF16, kind="ExternalInput").ap()
    wo_d = nc.dram_tensor("wo", [192, C], BF16, kind="ExternalInput").ap()
    y_d = nc.dram_tensor("y", [T, C], BF16, kind="ExternalOutput").ap()
    dbg = None
    if os.environ.get("KDBG"):
        dbg = {
            "es": nc.dram_tensor("dbg_es", [4, 128, 3, SW], BF16,
                                 kind="ExternalOutput").ap(),
            "qq": nc.dram_tensor("dbg_qq", [128, SW], BF16,
                                 kind="ExternalOutput").ap(),
            "qk2": nc.dram_tensor("dbg_qk2", [128, SW], BF16,
                                  kind="ExternalOutput").ap(),
            "kk": nc.dram_tensor("dbg_kk", [128, SW], BF16,
                                 kind="ExternalOutput").ap(),
            "k2": nc.dram_tensor("dbg_k2", [64, SW], BF16,
                                 kind="ExternalOutput").ap(),
            "vtm": nc.dram_tensor("dbg_vtm", [128, 4, D + 1], BF16,
                                  kind="ExternalOutput").ap(),
            "yacc": nc.dram_tensor("dbg_yacc", [3, 65, SW], F32,
                                   kind="ExternalOutput").ap(),
            "rlb": nc.dram_tensor("dbg_rlb", [3, 64, SW], F32,
                                  kind="ExternalOutput").ap(),
            "ya": nc.dram_tensor("dbg_ya", [128, SW], BF16,
                                 kind="ExternalOutput").ap(),
        }

    with tile.TileContext(nc) as tc, ExitStack() as ctx:
        kernel_body(tc, ctx, xT_d, wqk_d, wv_d, wo_d, y_d, dbg)
    nc.compile()
    return nc


def kernel_body(tc, ctx, xT_d, wqk_d, wv_d, wo_d, y_d, dbg=None):
    nc = tc.nc
    EXP = mybir.ActivationFunctionType.Exp
    dram_pool = ctx.enter_context(tc.tile_pool(name="dram", bufs=1, space="DRAM"))
    scratch_d = dram_pool.tile([NS, HPC, SW], F32, name="scratch")

    singles = ctx.enter_context(tc.tile_pool(name="singles", bufs=1))
    xt_pool = ctx.enter_context(tc.tile_pool(name="xt_pool", bufs=12))
    qq_pool = ctx.enter_context(tc.tile_pool(name="qq_pool", bufs=2))
    es_pool = ctx.enter_context(tc.tile_pool(name="es_pool", bufs=18))
    ya_pool = ctx.enter_context(tc.tile_pool(name="ya_pool", bufs=3))
    yst_pool = ctx.enter_context(tc.tile_pool(name="yst_pool", bufs=2))
    rl_pool = ctx.enter_context(tc.tile_pool(name="rl_pool", bufs=2))
    out_pool = ctx.enter_context(tc.tile_pool(name="out_pool", bufs=2))
    ps_s = ctx.enter_context(tc.tile_pool(name="ps_s", bufs=2, space="PSUM"))
    ps_y = ctx.enter_context(tc.tile_pool(name="ps_y", bufs=2, space="PSUM"))

    # ---- constants ----
    # w_qk as 6 contraction-chunk tiles [128, 384] (f-tiles [q0|q1],[k0|k1],[q2|k2])
    w_sb = []
    for kc in range(6):
        wt = singles.tile([128, 384], BF16, name=f"w_sb{kc}")
        nc.sync.dma_start(wt, wqk_d[kc * 128:(kc + 1) * 128, :])
        w_sb.append(wt)
    wv_sb = []
    for kc in range(6):
        wt = singles.tile([128, 192], BF16, name=f"wv_sb{kc}")
        nc.sync.dma_start(wt, wv_d[kc * 128:(kc + 1) * 128, :])
        wv_sb.append(wt)
    woA = singles.tile([128, C], BF16)
    nc.sync.dma_start(woA, wo_d[0:128, :])
    woB = singles.tile([64, C], BF16)
    nc.sync.dma_start(woB, wo_d[128:192, :])

    # single causal band mask: M[p, f] = 1 if f >= p else 0
    mask = singles.tile([128, 128], BF16, name="mask")
    nc.gpsimd.memset(mask, 1.0)
    nc.gpsimd.affine_select(
        out=mask, in_=mask, compare_op=mybir.AluOpType.is_ge, fill=0.0,
        base=0, pattern=[[1, 128]], channel_multiplier=-1)

    # resident K storage: KK[s] = [k0|k1], K2[s] = [k2|k2] (both halves, so
    # the h2 QK matmul can alternate PE row groups by tile parity and run
    # concurrently with the adjacent tile's other-half matmuls)
    KK = [singles.tile([128, SW], BF16, name=f"KK{s}") for s in range(NS)]
    K2 = [singles.tile([128, SW], BF16, name=f"K2{s}") for s in range(NS)]

    # token-major V with ones column per head, all 32 k-tiles
    vtm = [singles.tile([128, NKT, D + 1], BF16, name=f"vtm{h}") for h in range(HPC)]
    ones_col = singles.tile([128, NKT], BF16)
    nc.vector.memset(ones_col, 1.0)
    for h in range(HPC):
        nc.vector.tensor_copy(vtm[h][:, :, D:D + 1], ones_col.unsqueeze(2))

    qq_tiles = [None] * NS   # [q0|q1] per strip (strip-local lifetime)
    q2h_tiles = [None] * NS  # q2 duplicated at partitions 64:128
    pending = []             # deferred PV-chain emitters (cross-strip)

    # ---------------- Phase A for one strip (chunk generator) ----------------
    # Yields after each PE-dense chunk so phase_b can interleave these chunks
    # into its ACT-paced stretches, keeping the PE HAM-warm.
    def phase_a(s):
        xT = []
        for kc in range(6):
            xtc = xt_pool.tile([128, SW], BF16, name=f"xT_{s}_{kc}", tag="xT")
            nc.sync.dma_start(xtc, xT_d[kc * 128:(kc + 1) * 128,
                                        s * SW:(s + 1) * SW])
            xT.append(xtc)
        yield

        # projection f-tiles: [q0|q1],[k0|k1],[q2|k2]. Each f-tile gets its
        # own single-bank PSUM tile from the ps_y pool so filler projection
        # work never blocks on the score-pipeline (ACT-paced) ps_s pool.
        qq = qq_pool.tile([128, SW], BF16, name=f"qq_{s}", tag="qq")
        qk2 = qq_pool.tile([128, SW], BF16, name=f"qk2_{s}", tag="qq2")
        qq_tiles[s] = (qq, qk2)
        for ft, dest in enumerate([qq, KK[s], qk2]):
            psp = ps_y.tile([128, SW], F32, name=f"ps_pj_{s}_{ft}", tag="psy")
            for kc in range(6):
                nc.tensor.matmul(
                    psp,
                    w_sb[kc][:, ft * 128:(ft + 1) * 128],
                    xT[kc],
                    start=(kc == 0), stop=(kc == 5))
            nc.vector.tensor_copy(dest, psp)
            yield
        # duplicate k2 into both halves of resident K2[s]; duplicate q2 into
        # the upper half of a separate tile (no write into qk2 itself: DMAs
        # on the same queue can run concurrently, so an in-tile overwrite
        # would race the K2 reads) so h2 QK can run on either PE row group
        q2h = qq_pool.tile([128, SW], BF16, name=f"q2h_{s}", tag="q2h")
        q2h_tiles[s] = q2h
        nc.gpsimd.dma_start(K2[s][0:64, :], qk2[64:128, :])
        nc.gpsimd.dma_start(K2[s][64:128, :], qk2[64:128, :])
        nc.gpsimd.dma_start(q2h[64:128, :], qk2[0:64, :])
        if dbg is not None and s == 0:
            nc.sync.dma_start(dbg["qq"], qq)
            nc.sync.dma_start(dbg["qk2"], qk2)
            nc.sync.dma_start(dbg["kk"], KK[s])
            nc.sync.dma_start(dbg["k2"], K2[s])
        yield

        # V token-major directly: xT chunks as weights, stream w_v
        for tt in range(4):
            psv = ps_y.tile([128, 192], F32, name=f"ps_v_{s}_{tt}", tag="psy")
            for kc in range(6):
                nc.tensor.matmul(
                    psv,
                    xT[kc][:, tt * 128:(tt + 1) * 128],
                    wv_sb[kc],
                    start=(kc == 0), stop=(kc == 5))
            kt = 4 * s + tt
            for h in range(HPC):
                nc.vector.tensor_copy(vtm[h][:, kt, 0:D],
                                      psv[:, h * 64:(h + 1) * 64])
            if dbg is not None and s == 0:
                nc.sync.dma_start(dbg["vtm"][:, tt, :], vtm[0][:, kt, :])
            yield

    # ---------------- Phase B + C for one strip ----------------
    def phase_b(s, fillers=None, last=False):
        nkt = 4 * s + 4                   # causal k tiles for this strip
        fillers = list(fillers or [])

        def fill_one():
            while fillers:
                g = fillers.pop(0)
                try:
                    next(g)
                except StopIteration:
                    continue
                fillers.append(g)
                return

        qq, qk2 = qq_tiles[s]
        q2h = q2h_tiles[s]
        # accumulators in SBUF per head: [65, 512] (row 64 = l)
        yacc = [ya_pool.tile([65, SW], F32, name=f"yacc_{s}_{h}", tag=f"yacc{h}")
                for h in range(HPC)]

        groups = [list(range(g, min(g + KG, nkt))) for g in range(0, nkt, KG)]

        def make_pv(gi, grp, es_tiles):
            def emit(h, gi=gi, grp=grp, es_tiles=es_tiles):
                psy = ps_y.tile([65, SW], F32, name=f"ps_y_{s}_{gi}_{h}",
                                tag="psy")
                for u, i in enumerate(grp):
                    j = i - 4 * s
                    c0 = 128 * j if j >= 0 else 0
                    nc.tensor.matmul(
                        psy[:, c0:], vtm[h][:, i, :],
                        es_tiles[i][:, h, c0:],
                        start=(u == 0), stop=(u == len(grp) - 1))
                if gi == 0:
                    nc.vector.tensor_copy(yacc[h], psy)
                else:
                    nc.vector.tensor_add(yacc[h], yacc[h], psy)
            return [lambda h=h: emit(h) for h in range(HPC)]

        def slot():
            # drain ALL pending chains before any filler: epilogue fillers
            # read the yacc tiles that these chains initialize, and a filler
            # chunk may touch all three heads at once
            while pending:
                pending.pop(0)()
            fill_one()

        for gi, grp in enumerate(groups):
            es_tiles = {}
            for u, i in enumerate(grp):
                st = KK[i // 4]
                k2t = K2[i // 4]
                sl = slice((i % 4) * 128, (i % 4) * 128 + 128)
                j = i - 4 * s
                c0 = 128 * j if j >= 0 else 0
                pss = ps_s.tile([128, 3, SW], F32, name=f"ps_s_{s}_{i}", tag="S")
                # h2 alternates PE row halves by tile parity and is issued
                # adjacent to the other parity's h2 so the two overlap; h0/h1
                # always pair (disjoint row groups, concurrent starts).
                h2 = (pss[:, 2, c0:], k2t[0:64, sl], qk2[0:64, c0:]) \
                    if i % 2 == 0 else \
                     (pss[:, 2, c0:], k2t[64:128, sl], q2h[64:128, c0:])
                if i % 2 == 0:
                    nc.tensor.matmul(pss[:, 0, c0:], st[0:64, sl],
                                     qq[0:64, c0:], start=True, stop=True)
                    nc.tensor.matmul(pss[:, 1, c0:], st[64:128, sl],
                                     qq[64:128, c0:], start=True, stop=True)
                    nc.tensor.matmul(h2[0], h2[1], h2[2], start=True, stop=True)
                else:
                    nc.tensor.matmul(h2[0], h2[1], h2[2], start=True, stop=True)
                    nc.tensor.matmul(pss[:, 0, c0:], st[0:64, sl],
                                     qq[0:64, c0:], start=True, stop=True)
                    nc.tensor.matmul(pss[:, 1, c0:], st[64:128, sl],
                                     qq[64:128, c0:], start=True, stop=True)
                es = es_pool.tile([128, 3, SW], BF16, name=f"es_{s}_{i}", tag="es")
                nc.scalar.activation(es[:, :, c0:], pss[:, :, c0:], EXP,
                                     scale=0.125)
                if j >= 0:  # diagonal tile: apply causal mask to the band
                    for h in range(HPC):
                        nc.vector.tensor_mul(
                            es[:, h, c0:c0 + 128],
                            es[:, h, c0:c0 + 128], mask)
                es_tiles[i] = es
                if dbg is not None and s == 0:
                    nc.sync.dma_start(dbg["es"][i], es)
                if u % 2 == 1:   # only after odd tiles: keeps the h2-parity
                    slot()       # matmuls adjacent so they pair on the PE
            slot()
            pending.extend(make_pv(gi, grp, es_tiles))
        while pending:               # drain the last group's PV chains
            pending.pop(0)()
        while fillers:               # drain remaining filler chunks
            fill_one()
        return yacc

    # ---- strip epilogue: normalize + output projection (deferred) ----
    def epilogue(s, yacc):
        # generator: yields between chunks for interleaving into phase_b
        # reciprocal of l (in place, partition 64), bounce through DRAM to
        # broadcast across partitions 0:64 where each head's Y^T lives.
        # pack the 3 l rows into one [3, 512] tile (SBUF->SBUF partition
        # shift), one reciprocal for all heads, bounce through DRAM to
        # broadcast across partitions 0:64 where each head's Y^T lives.
        l3 = rl_pool.tile([HPC, SW], F32, name=f"l3_{s}", tag="l3")
        for h in range(HPC):
            if dbg is not None and s == 0:
                nc.sync.dma_start(dbg["yacc"][h], yacc[h])
            nc.gpsimd.dma_start(l3[h:h + 1, :], yacc[h][64:65, :])
        yield
        nc.vector.reciprocal(l3, l3)
        nc.gpsimd.dma_start(scratch_d[s], l3)
        yield
        rlb = []
        for h in range(HPC):
            rb = rl_pool.tile([64, SW], F32, name=f"rlb_{s}_{h}", tag=f"rlb{h}")
            nc.gpsimd.dma_start(
                rb, scratch_d[s, h, :].unsqueeze(0).to_broadcast((64, SW)))
            rlb.append(rb)
            yield

        # normalized, stacked Y^T: ya[0:64] = h0, ya[64:128] = h1 (DMA shift)
        ya = yst_pool.tile([128, SW], BF16, name=f"ya_{s}", tag="ya")
        y2 = yst_pool.tile([64, SW], BF16, name=f"y2_{s}", tag="y2")
        ytmp = yst_pool.tile([64, SW], BF16, name=f"ytmp_{s}", tag="ytmp")
        nc.vector.tensor_mul(ya[0:64, :], yacc[0][0:64, :], rlb[0])
        nc.vector.tensor_mul(ytmp, yacc[1][0:64, :], rlb[1])
        nc.vector.tensor_mul(y2, yacc[2][0:64, :], rlb[2])
        nc.gpsimd.dma_start(ya[64:128, :], ytmp)
        if dbg is not None and s == 0:
            for h in range(HPC):
                nc.sync.dma_start(dbg["rlb"][h], rlb[h])
            nc.sync.dma_start(dbg["ya"], ya)
        yield

        # out projection per 128-q tile: out = ya.T @ woA + y2.T @ woB.
        # Single-bank PSUM tiles (ps_y pool) so this filler work never blocks
        # on the ACT-paced ps_s pool.
        for qt in range(4):
            osb = out_pool.tile([128, C], BF16, name=f"osb_{s}_{qt}", tag="osb")
            qsl = slice(qt * 128, (qt + 1) * 128)
            for (n0, n1) in ((0, 512), (512, 768)):
                pso = ps_y.tile([128, n1 - n0], F32,
                                name=f"ps_o_{s}_{qt}_{n0}", tag="psy")
                nc.tensor.matmul(pso, ya[:, qsl],
                                 woA[:, n0:n1], start=True, stop=False)
                nc.tensor.matmul(pso, y2[:, qsl],
                                 woB[:, n0:n1], start=False, stop=True)
                nc.vector.tensor_copy(osb[:, n0:n1], pso)
            nc.sync.dma_start(y_d[s * SW + qt * 128: s * SW + (qt + 1) * 128, :],
                              osb)
            if qt < 3:
                yield

    ns_run = int(os.environ.get("KNS", str(NS)))
    for _ in phase_a(0):
        pass
    prev = None
    for s in range(ns_run):
        fillers = []
        if prev is not None:   # strip s-1 epilogue interleaves into s
            fillers.append(epilogue(s - 1, prev))
        if s + 1 < ns_run:
            fillers.append(phase_a(s + 1))
        prev = phase_b(s, fillers, last=(s == ns_run - 1))
    for _ in epilogue(ns_run - 1, prev):
        pass


_PROGRAM_CACHE = {}


def _get_program():
    if "nc" not in _PROGRAM_CACHE:
        _PROGRAM_CACHE["nc"] = build_program()
    return _PROGRAM_CACHE["nc"]


def make_in_maps(x, w_qkv, w_out):
    bf = ml_dtypes.bfloat16
    x = np.asarray(x, dtype=np.float32)
    w_qkv = np.asarray(w_qkv, dtype=np.float32)
    w_out = np.asarray(w_out, dtype=np.float32)
    in_maps = []
    for c in range(8):
        b, g = c // 4, c % 4
        q = w_qkv[:, 192 * g: 192 * g + 192]
        k = w_qkv[:, 768 + 192 * g: 768 + 192 * g + 192]
        v = w_qkv[:, 1536 + 192 * g: 1536 + 192 * g + 192]
        wqk = np.concatenate(
            [q[:, 0:128], k[:, 0:128], q[:, 128:192], k[:, 128:192]], axis=1)
        in_maps.append({
            "xT": np.ascontiguousarray(x[b].T).astype(bf),
            "wqk": np.ascontiguousarray(wqk).astype(bf),
            "wv": np.ascontiguousarray(v).astype(bf),
            "wo": np.ascontiguousarray(w_out[192 * g: 192 * g + 192]).astype(bf),
        })
    return in_maps


def kernel(x, w_qkv, w_out, trace=False):
    nc = _get_program()
    in_maps = make_in_maps(x, w_qkv, w_out)
    res = run_bass_kernel_spmd(nc, in_maps, list(range(8)), trace=trace)
    out = np.zeros((B, T, C), dtype=np.float32)
    for c in range(8):
        out[c // 4] += np.asarray(res.results[c]["y"], dtype=np.float32)
    kernel.last_result = res
    return out


# revision 46
# speedup vs baseline: 1.0044x; 1.0044x over previous
"""Causal self-attention (B=2, T=4096, C=768, H=12, D=64) on 8 trn2 cores.

Sharding: core c handles batch b = c//4 and heads [3g, 3g+3), g = c%4.
Each core computes a (4096, 768) partial of y = attn_out @ w_out restricted
to its 3 heads' rows of w_out; the host sums the 4 partials per batch.

v2: all matmul operands bf16 (f32 PSUM), x pre-transposed + pre-cast on the
host (no PE transposes), V computed token-major directly (xT chunks as
weights streaming w_v), causal column-trim on QK/exp/PV for diagonal tiles
with a single [128,128] triangular mask applied to the diagonal band on DVE,
reciprocal_approx_fast for softmax denominators.

Math per head (no max-subtraction softmax; scores are O(8) so exp is safe):
  S^T[k, q] = (K Q^T)[k, q] / 8     computed k-on-partitions
  E = exp(S^T) with causal band mask
  [Y^T; l] = [V | 1]^T E            PV matmul with a ones column -> row 64 = l
  out += (Y^T / l).T @ W_o[head rows]
"""

import os
import numpy as np
import ml_dtypes
from contextlib import ExitStack

import concourse.bass as bass
import concourse.tile as tile
from concourse import bacc, mybir
from concourse.bass_utils import run_bass_kernel_spmd

F32 = mybir.dt.float32
BF16 = mybir.dt.bfloat16

B, T, C, H, D = 2, 4096, 768, 12, 64
HPC = 3            # heads per core
NS = 8             # strips
SW = 512           # strip width (q)
KT = 128           # k tile
NKT = T // KT      # 32 k tiles
KG = 4             # k tiles per PV accumulation group


def build_program():
    nc = bacc.Bacc("TRN2", target_bir_lowering=False, debug=False, num_devices=8)

    xT_d = nc.dram_tensor("xT", [C, T], BF16, kind="ExternalInput").ap()
    wqk_d = nc.dram_tensor("wqk", [C, 384], BF16, kind="ExternalInput").ap()
    wv_d = nc.dram_tensor("wv", [C, 192], BF16, kind="ExternalInput").ap()
    wo_d = nc.dram_tensor("wo", [192, C], BF16, kind="ExternalInput").ap()
    y_d = nc.dram_tensor("y", [T, C], BF16, kind="ExternalOutput").ap()
    dbg = None
    if os.environ.get("KDBG"):
        dbg = {
            "es": nc.dram_tensor("dbg_es", [4, 128, 3, SW], BF16,
                                 kind="ExternalOutput").ap(),
            "qq": nc.dram_tensor("dbg_qq", [128, SW], BF16,
                                 kind="ExternalOutput").ap(),
            "qk2": nc.dram_tensor("dbg_qk2", [128, SW], BF16,
                                  kind="ExternalOutput").ap(),
            "kk": nc.dram_tensor("dbg_kk", [128, SW], BF16,
                                 kind="ExternalOutput").ap(),
            "k2": nc.dram_tensor("dbg_k2", [64, SW], BF16,
                                 kind="ExternalOutput").ap(),
            "vtm": nc.dram_tensor("dbg_vtm", [128, 4, D + 1], BF16,
                                  kind="ExternalOutput").ap(),
            "yacc": nc.dram_tensor("dbg_yacc", [3, 65, SW], F32,
                                   kind="ExternalOutput").ap(),
            "rlb": nc.dram_tensor("dbg_rlb", [3, 64, SW], F32,
                                  kind="ExternalOutput").ap(),
            "ya": nc.dram_tensor("dbg_ya", [128, SW], BF16,
                                 kind="ExternalOutput").ap(),
        }

    with tile.TileContext(nc) as tc, ExitStack() as ctx:
        kernel_body(tc, ctx, xT_d, wqk_d, wv_d, wo_d, y_d, dbg)
    nc.compile()
    return nc


def kernel_body(tc, ctx, xT_d, wqk_d, wv_d, wo_d, y_d, dbg=None):
    nc = tc.nc
    EXP = mybir.ActivationFunctionType.Exp
    dram_pool = ctx.enter_context(tc.tile_pool(name="dram", bufs=1, space="DRAM"))
    scratch_d = dram_pool.tile([NS, HPC, SW], F32, name="scratch")

    singles = ctx.enter_context(tc.tile_pool(name="singles", bufs=1))
    xt_pool = ctx.enter_context(tc.tile_pool(name="xt_pool", bufs=12))
    qq_pool = ctx.enter_context(tc.tile_pool(name="qq_pool", bufs=2))
    es_pool = ctx.enter_context(tc.tile_pool(name="es_pool", bufs=10))
    ya_pool = ctx.enter_context(tc.tile_pool(name="ya_pool", bufs=3))
    yst_pool = ctx.enter_context(tc.tile_pool(name="yst_pool", bufs=2))
    rl_pool = ctx.enter_context(tc.tile_pool(name="rl_pool", bufs=2))
    out_pool = ctx.enter_context(tc.tile_pool(name="out_pool", bufs=2))
    ps_s = ctx.enter_context(tc.tile_pool(name="ps_s", bufs=2, space="PSUM"))
    ps_y = ctx.enter_context(tc.tile_pool(name="ps_y", bufs=2, space="PSUM"))

    # ---- constants ----
    # w_qk as 6 contraction-chunk tiles [128, 384] (f-tiles [q0|q1],[k0|k1],[q2|k2])
    w_sb = []
    for kc in range(6):
        wt = singles.tile([128, 384], BF16, name=f"w_sb{kc}")
        nc.sync.dma_start(wt, wqk_d[kc * 128:(kc + 1) * 128, :])
        w_sb.append(wt)
    wv_sb = []
    for kc in range(6):
        wt = singles.tile([128, 192], BF16, name=f"wv_sb{kc}")
        nc.sync.dma_start(wt, wv_d[kc * 128:(kc + 1) * 128, :])
        wv_sb.append(wt)
    woA = singles.tile([128, C], BF16)
    nc.sync.dma_start(woA, wo_d[0:128, :])
    woB = singles.tile([64, C], BF16)
    nc.sync.dma_start(woB, wo_d[128:192, :])

    # single causal band mask: M[p, f] = 1 if f >= p else 0
    mask = singles.tile([128, 128], BF16, name="mask")
    nc.gpsimd.memset(mask, 1.0)
    nc.gpsimd.affine_select(
        out=mask, in_=mask, compare_op=mybir.AluOpType.is_ge, fill=0.0,
        base=0, pattern=[[1, 128]], channel_multiplier=-1)

    # resident K storage: KK[s] = [k0|k1], K2[s] = [k2|k2] (both halves, so
    # the h2 QK matmul can alternate PE row groups by tile parity and run
    # concurrently with the adjacent tile's other-half matmuls)
    KK = [singles.tile([128, SW], BF16, name=f"KK{s}") for s in range(NS)]
    K2 = [singles.tile([128, SW], BF16, name=f"K2{s}") for s in range(NS)]

    # token-major V with ones column per head, all 32 k-tiles
    vtm = [singles.tile([128, NKT, D + 1], BF16, name=f"vtm{h}") for h in range(HPC)]
    ones_col = singles.tile([128, NKT], BF16)
    nc.vector.memset(ones_col, 1.0)
    for h in range(HPC):
        nc.vector.tensor_copy(vtm[h][:, :, D:D + 1], ones_col.unsqueeze(2))

    qq_tiles = [None] * NS   # [q0|q1] per strip (strip-local lifetime)
    q2h_tiles = [None] * NS  # q2 duplicated at partitions 64:128
    pending = []             # deferred PV-chain emitters (cross-strip)

    # ---------------- Phase A for one strip (chunk generator) ----------------
    # Yields after each PE-dense chunk so phase_b can interleave these chunks
    # into its ACT-paced stretches, keeping the PE HAM-warm.
    def phase_a(s):
        xT = []
        for kc in range(6):
            xtc = xt_pool.tile([128, SW], BF16, name=f"xT_{s}_{kc}", tag="xT")
            nc.sync.dma_start(xtc, xT_d[kc * 128:(kc + 1) * 128,
                                        s * SW:(s + 1) * SW])
            xT.append(xtc)
        yield

        # projection f-tiles: [q0|q1],[k0|k1],[q2|k2]. Each f-tile gets its
        # own single-bank PSUM tile from the ps_y pool so filler projection
        # work never blocks on the score-pipeline (ACT-paced) ps_s pool.
        qq = qq_pool.tile([128, SW], BF16, name=f"qq_{s}", tag="qq")
        qk2 = qq_pool.tile([128, SW], BF16, name=f"qk2_{s}", tag="qq2")
        qq_tiles[s] = (qq, qk2)
        for ft, dest in enumerate([qq, KK[s], qk2]):
            psp = ps_y.tile([128, SW], F32, name=f"ps_pj_{s}_{ft}", tag="psy")
            for kc in range(6):
                nc.tensor.matmul(
                    psp,
                    w_sb[kc][:, ft * 128:(ft + 1) * 128],
                    xT[kc],
                    start=(kc == 0), stop=(kc == 5))
            nc.vector.tensor_copy(dest, psp)
            yield
        # duplicate k2 into both halves of resident K2[s]; duplicate q2 into
        # the upper half of a separate tile (no write into qk2 itself: DMAs
        # on the same queue can run concurrently, so an in-tile overwrite
        # would race the K2 reads) so h2 QK can run on either PE row group
        q2h = qq_pool.tile([128, SW], BF16, name=f"q2h_{s}", tag="q2h")
        q2h_tiles[s] = q2h
        nc.gpsimd.dma_start(K2[s][0:64, :], qk2[64:128, :])
        nc.gpsimd.dma_start(K2[s][64:128, :], qk2[64:128, :])
        nc.gpsimd.dma_start(q2h[64:128, :], qk2[0:64, :])
        if dbg is not None and s == 0:
            nc.sync.dma_start(dbg["qq"], qq)
            nc.sync.dma_start(dbg["qk2"], qk2)
            nc.sync.dma_start(dbg["kk"], KK[s])
            nc.sync.dma_start(dbg["k2"], K2[s])
        yield

        # V token-major directly: xT chunks as weights, stream w_v
        for tt in range(4):
            psv = ps_y.tile([128, 192], F32, name=f"ps_v_{s}_{tt}", tag="psy")
            for kc in range(6):
                nc.tensor.matmul(
                    psv,
                    xT[kc][:, tt * 128:(tt + 1) * 128],
                    wv_sb[kc],
                    start=(kc == 0), stop=(kc == 5))
            kt = 4 * s + tt
            for h in range(HPC):
                nc.vector.tensor_copy(vtm[h][:, kt, 0:D],
                                      psv[:, h * 64:(h + 1) * 64])
            if dbg is not None and s == 0:
                nc.sync.dma_start(dbg["vtm"][:, tt, :], vtm[0][:, kt, :])
            yield

    # ---------------- Phase B + C for one strip ----------------
    def phase_b(s, fillers=None, last=False):
        nkt = 4 * s + 4                   # causal k tiles for this strip
        fillers = list(fillers or [])

        def fill_one():
            while fillers:
                g = fillers.pop(0)
                try:
                    next(g)
                except StopIteration:
                    continue
                fillers.append(g)
                return

        qq, qk2 = qq_tiles[s]
        q2h = q2h_tiles[s]
        # accumulators in SBUF per head: [65, 512] (row 64 = l)
        yacc = [ya_pool.tile([65, SW], F32, name=f"yacc_{s}_{h}", tag=f"yacc{h}")
                for h in range(HPC)]

        groups = [list(range(g, min(g + KG, nkt))) for g in range(0, nkt, KG)]

        def make_pv(gi, grp, es_tiles):
            def emit(h, gi=gi, grp=grp, es_tiles=es_tiles):
                psy = ps_y.tile([65, SW], F32, name=f"ps_y_{s}_{gi}_{h}",
                                tag="psy")
                for u, i in enumerate(grp):
                    j = i - 4 * s
                    c0 = 128 * j if j >= 0 else 0
                    nc.tensor.matmul(
                        psy[:, c0:], vtm[h][:, i, :],
                        es_tiles[i][:, h, c0:],
                        start=(u == 0), stop=(u == len(grp) - 1))
                if gi == 0:
                    nc.vector.tensor_copy(yacc[h], psy)
                else:
                    nc.vector.tensor_add(yacc[h], yacc[h], psy)
            return [lambda h=h: emit(h) for h in range(HPC)]

        def slot():
            # drain ALL pending chains before any filler: epilogue fillers
            # read the yacc tiles that these chains initialize, and a filler
            # chunk may touch all three heads at once
            while pending:
                pending.pop(0)()
            fill_one()

        for gi, grp in enumerate(groups):
            es_tiles = {}
            for u, i in enumerate(grp):
                st = KK[i // 4]
                k2t = K2[i // 4]
                sl = slice((i % 4) * 128, (i % 4) * 128 + 128)
                j = i - 4 * s
                c0 = 128 * j if j >= 0 else 0
                pss = ps_s.tile([128, 3, SW], F32, name=f"ps_s_{s}_{i}", tag="S")
                # h2 alternates PE row halves by tile parity and is issued
                # adjacent to the other parity's h2 so the two overlap; h0/h1
                # always pair (disjoint row groups, concurrent starts).
                h2 = (pss[:, 2, c0:], k2t[0:64, sl], qk2[0:64, c0:]) \
                    if i % 2 == 0 else \
                     (pss[:, 2, c0:], k2t[64:128, sl], q2h[64:128, c0:])
                if i % 2 == 0:
                    nc.tensor.matmul(pss[:, 0, c0:], st[0:64, sl],
                                     qq[0:64, c0:], start=True, stop=True)
                    nc.tensor.matmul(pss[:, 1, c0:], st[64:128, sl],
                                     qq[64:128, c0:], start=True, stop=True)
                    nc.tensor.matmul(h2[0], h2[1], h2[2], start=True, stop=True)
                else:
                    nc.tensor.matmul(h2[0], h2[1], h2[2], start=True, stop=True)
                    nc.tensor.matmul(pss[:, 0, c0:], st[0:64, sl],
                                     qq[0:64, c0:], start=True, stop=True)
                    nc.tensor.matmul(pss[:, 1, c0:], st[64:128, sl],
                                     qq[64:128, c0:], start=True, stop=True)
                es = es_pool.tile([128, 3, SW], BF16, name=f"es_{s}_{i}", tag="es")
                nc.scalar.activation(es[:, :, c0:], pss[:, :, c0:], EXP,
                                     scale=0.125)
                if j >= 0:  # diagonal tile: apply causal mask to the band
                    for h in range(HPC):
                        nc.vector.tensor_mul(
                            es[:, h, c0:c0 + 128],
                            es[:, h, c0:c0 + 128], mask)
                es_tiles[i] = es
                if dbg is not None and s == 0:
                    nc.sync.dma_start(dbg["es"][i], es)
                if u % 2 == 1:   # only after odd tiles: keeps the h2-parity
                    slot()       # matmuls adjacent so they pair on the PE
            slot()
            pending.extend(make_pv(gi, grp, es_tiles))
        while pending:               # drain the last group's PV chains
            pending.pop(0)()
        while fillers:               # drain remaining filler chunks
            fill_one()
        return yacc

    # ---- strip epilogue: normalize + output projection (deferred) ----
    def epilogue(s, yacc):
        # generator: yields between chunks for interleaving into phase_b
        # reciprocal of l (in place, partition 64), bounce through DRAM to
        # broadcast across partitions 0:64 where each head's Y^T lives.
        # pack the 3 l rows into one [3, 512] tile (SBUF->SBUF partition
        # shift), one reciprocal for all heads, bounce through DRAM to
        # broadcast across partitions 0:64 where each head's Y^T lives.
        l3 = rl_pool.tile([HPC, SW], F32, name=f"l3_{s}", tag="l3")
        for h in range(HPC):
            if dbg is not None and s == 0:
                nc.sync.dma_start(dbg["yacc"][h], yacc[h])
            nc.gpsimd.dma_start(l3[h:h + 1, :], yacc[h][64:65, :])
        yield
        nc.vector.reciprocal(l3, l3)
        nc.gpsimd.dma_start(scratch_d[s], l3)
        yield
        rlb = []
        for h in range(HPC):
            rb = rl_pool.tile([64, SW], F32, name=f"rlb_{s}_{h}", tag=f"rlb{h}")
            nc.gpsimd.dma_start(
                rb, scratch_d[s, h, :].unsqueeze(0).to_broadcast((64, SW)))
            rlb.append(rb)
            yield

        # normalized, stacked Y^T: ya[0:64] = h0, ya[64:128] = h1 (DMA shift)
        ya = yst_pool.tile([128, SW], BF16, name=f"ya_{s}", tag="ya")
        y2 = yst_pool.tile([64, SW], BF16, name=f"y2_{s}", tag="y2")
        ytmp = yst_pool.tile([64, SW], BF16, name=f"ytmp_{s}", tag="ytmp")
        nc.vector.tensor_mul(ya[0:64, :], yacc[0][0:64, :], rlb[0])
        nc.vector.tensor_mul(ytmp, yacc[1][0:64, :], rlb[1])
        nc.vector.tensor_mul(y2, yacc[2][0:64, :], rlb[2])
        nc.gpsimd.dma_start(ya[64:128, :], ytmp)
        if dbg is not None and s == 0:
            for h in range(HPC):
                nc.sync.dma_start(dbg["rlb"][h], rlb[h])
            nc.sync.dma_start(dbg["ya"], ya)
        yield

        # out projection per 128-q tile: out = ya.T @ woA + y2.T @ woB.
        # Single-bank PSUM tiles (ps_y pool) so this filler work never blocks
        # on the ACT-paced ps_s pool.
        for qt in range(4):
            osb = out_pool.tile([128, C], BF16, name=f"osb_{s}_{qt}", tag="osb")
            qsl = slice(qt * 128, (qt + 1) * 128)
            for (n0, n1) in ((0, 512), (512, 768)):
                pso = ps_y.tile([128, n1 - n0], F32,
                                name=f"ps_o_{s}_{qt}_{n0}", tag="psy")
                nc.tensor.matmul(pso, ya[:, qsl],
                                 woA[:, n0:n1], start=True, stop=False)
                nc.tensor.matmul(pso, y2[:, qsl],
                                 woB[:, n0:n1], start=False, stop=True)
                nc.vector.tensor_copy(osb[:, n0:n1], pso)
            nc.sync.dma_start(y_d[s * SW + qt * 128: s * SW + (qt + 1) * 128, :],
                              osb)
            if qt < 3:
                yield

    ns_run = int(os.environ.get("KNS", str(NS)))
    for _ in phase_a(0):
        pass
    prev = None
    for s in range(ns_run):
        fillers = []
        if prev is not None:   # strip s-1 epilogue interleaves into s
            fillers.append(epilogue(s - 1, prev))
        if s + 1 < ns_run:
            fillers.append(phase_a(s + 1))
        prev = phase_b(s, fillers, last=(s == ns_run - 1))
    for _ in epilogue(ns_run - 1, prev):
        pass


_PROGRAM_CACHE = {}


def _get_program():
    if "nc" not in _PROGRAM_CACHE:
        _PROGRAM_CACHE["nc"] = build_program()
    return _PROGRAM_CACHE["nc"]


def make_in_maps(x, w_qkv, w_out):
    bf = ml_dtypes.bfloat16
    x = np.asarray(x, dtype=np.float32)
    w_qkv = np.asarray(w_qkv, dtype=np.float32)
    w_out = np.asarray(w_out, dtype=np.float32)
    in_maps = []
    for c in range(8):
        b, g = c // 4, c % 4
        q = w_qkv[:, 192 * g: 192 * g + 192]
        k = w_qkv[:, 768 + 192 * g: 768 + 192 * g + 192]
        v = w_qkv[:, 1536 + 192 * g: 1536 + 192 * g + 192]
        wqk = np.concatenate(
            [q[:, 0:128], k[:, 0:128], q[:, 128:192], k[:, 128:192]], axis=1)
        in_maps.append({
            "xT": np.ascontiguousarray(x[b].T).astype(bf),
            "wqk": np.ascontiguousarray(wqk).astype(bf),
            "wv": np.ascontiguousarray(v).astype(bf),
            "wo": np.ascontiguousarray(w_out[192 * g: 192 * g + 192]).astype(bf),
        })
    return in_maps


def kernel(x, w_qkv, w_out, trace=False):
    nc = _get_program()
    in_maps = make_in_maps(x, w_qkv, w_out)
    res = run_bass_kernel_spmd(nc, in_maps, list(range(8)), trace=trace)
    out = np.zeros((B, T, C), dtype=np.float32)
    for c in range(8):
        out[c // 4] += np.asarray(res.results[c]["y"], dtype=np.float32)
    kernel.last_result = res
    return out
